# revision 41
# baseline (speedup 1.0000x reference)
"""Trainium2 Bass kernel for batched uniform cubic B-spline evaluation.

Reference: out[b,i,o,e] = sum_c cp_pad[i,o,c] * N_c(x[b,i,e]) where N_c is the
cardinal cubic B-spline basis on uniform knots t = arange(-3,18)/14 and cp_pad
repeats the last control point twice (c = 0..17).

Formulation used here: N_c(x) = B3(u), u = 14x + 3 - c, and with a = |u - 2|:

    6*B3 = M(a) = relu(2-a)^3 - 4*relu(1-a)^3        (no cancellation, M in [0,4])

so out[i,o,e] = sum_c (cp_pad[i,o,c]/6) * M(|14x - (c+1)|) — a single fp16
matmul per i with K=18 (padded to 32-row strips, 4 i per 128 partitions).

Per core (batch b = core id), per group of 4 i's:
  1. one K=8 fp16 matmul broadcasts 14*(xh+xl) into the 4 strips (PSUM, exact)
  2. ACT: a = Abs(xb + bias_p) with per-partition bias -(c+1); pad rows c>=18
     get a >= 5 so M = 0 automatically
  3. two 1-uop custom DVE ops: t2 = sq(relu(2*(1-a)))*(1-a) = 4*relu(1-a)^3,
     M16 = sq(relu(2-a))*(2-a) - t2 -> fp16
  4. 4 fp16 matmuls (one per i, quadrant tile_position) into PSUM banks packed
     2 i's per bank; PSUM->SBUF copies rotate vector/scalar/gpsimd
  5. batched 1MB output DMAs (8 i's each)
"""

import numpy as np

B, ID, OD, NE, NCP = 8, 128, 128, 256, 16
NCORES = 8
STRIP = 32
NC18 = 18          # control points after padding (c = 0..17)

_cache = {}
_OUT_MODE = "sbuf"          # "sbuf_per_i" (contiguous 128KB per i) or "sbuf"


def _register_dve_ops():
    """Register the two 1-uop bump ops in dve_ops' registries (idempotent)."""
    if "dve" in _cache:
        return _cache["dve"]
    import concourse.dve_ops as dve_ops
    from concourse.dve_ops import DveOp
    from concourse.dve_spec import Spec, Src0, Src1, C0, C2, Zero, relu, sq, maxx

    # a = |in0 + c0| computed inside both ops (c0 = per-partition bias AP)
    V = Src0 + C0
    A = maxx(V, Zero - V)

    def _ref_t2x(in0, in1, c0, c1, c2):
        a = np.abs(in0.astype(np.float32) + c0)
        zm = c2 - a
        return (np.maximum(zm + zm, 0) ** 2 * zm).astype(np.float32)

    zm = C2 - A
    T2 = DveOp(
        "ANT_BUMP_T2X",
        Spec(body=sq(relu(zm + zm)) * zm, reference=_ref_t2x),
        subdim=False,
        uops_sha={"v3": "bf2265f9ea4d409b"},
    )

    def _ref_mx(in0, in1, c0, c1, c2):
        a = np.abs(in0.astype(np.float32) + c0)
        z = c2 - a
        return (np.maximum(z, 0) ** 2 * z - in1).astype(np.float32)

    z = C2 - A
    M = DveOp(
        "ANT_BUMP_MX",
        Spec(body=sq(relu(z)) * z - Src1, reference=_ref_mx),
        subdim=False,
        uops_sha={"v3": "e03e3c990f1a886b"},
    )

    for op in (T2, M):
        if op.name not in dve_ops._SUB_OPCODE_FOR_NAME:
            dve_ops.OPS.append(op)
            dve_ops._SUB_OPCODE_FOR_NAME[op.name] = (
                max(dve_ops._SUB_OPCODE_FOR_NAME.values()) + 1
            )
            dve_ops.CUSTOM_DVE_SPECS[op.name] = op.spec
    assert max(dve_ops._SUB_OPCODE_FOR_NAME.values()) < 0x20
    _cache["dve"] = (T2, M)
    return T2, M


def _build_program():
    import concourse.mybir as mybir
    import concourse.tile as tile
    from concourse import bacc

    T2OP, MOP = _register_dve_ops()

    F32 = mybir.dt.float32
    F16 = mybir.dt.float16
    ABS = mybir.ActivationFunctionType.Abs

    from concourse.alu_op_type import AluOpType

    nc = bacc.Bacc("TRN2", target_bir_lowering=False)
    w_d = nc.dram_tensor("w", [128, 32 * 128], F16, kind="ExternalInput")
    xhm_d = nc.dram_tensor("xhm", [128, 512], F16, kind="ExternalInput")
    selw_d = nc.dram_tensor("selw", [128, 512], F16, kind="ExternalInput")
    bias_d = nc.dram_tensor("bias", [128, 1], F32, kind="ExternalInput")
    # fp16 output in [o, i, e] layout: halves write traffic and gives 4KB
    # contiguous DMA descriptors; host transposes + upcasts to fp32.
    out_d = nc.dram_tensor("out", [128, 128, 256], F16, kind="ExternalOutput")

    with tile.TileContext(nc) as tc:
        with (
            tc.tile_pool(name="const", bufs=1) as cpool,
            tc.tile_pool(name="work", bufs=5) as pool,
            tc.tile_pool(name="obp", bufs=4) as obpool,
            tc.tile_pool(name="xbp", bufs=1, space="PSUM") as xbpool,
            tc.tile_pool(name="mmp", bufs=1, space="PSUM") as mmpool,
        ):
            # few big DMAs: SP dispatch is ~600ns per dma_start
            selw_t = cpool.tile([128, 512], F16)
            nc.sync.dma_start(out=selw_t[:], in_=selw_d.ap())
            bias_t = cpool.tile([128, 1], F32)
            nc.sync.dma_start(out=bias_t[:], in_=bias_d.ap())
            xhm_t = cpool.tile([128, 512], F16)
            nc.sync.dma_start(out=xhm_t[:], in_=xhm_d.ap())
            w_t = cpool.tile([128, 32 * 128], F16)
            nc.sync.dma_start(out=w_t[:, 0:1024], in_=w_d.ap()[:, 0:1024])
            nc.sync.dma_start(out=w_t[:, 1024:4096],
                              in_=w_d.ap()[:, 1024:4096])

            ncopy = 0
            ob = None
            for grp in range(32):
                q, s, fcb = grp % 4, (grp // 4) % 4, grp // 16
                pr, fc = 32 * q + 8 * s, 256 * fcb

                xb = xbpool.tile([128, 256], F32, tag=f"xb{grp % 2}",
                                 name=f"xb_{grp}")
                nc.tensor.matmul(
                    xb[:], selw_t[32 * q:32 * q + 32, 128 * s:128 * s + 128],
                    xhm_t[32 * q:32 * q + 32, fc:fc + 256],
                    start=True, stop=True, tile_position=(32 * q, 0),
                )
                t2_t = pool.tile([128, 256], F32, tag="t2", name=f"t2_{grp}")
                nc.vector._custom_dve(T2OP, out=t2_t[:], in0=xb[:],
                                      s0=bias_t[:], imm2=1.0)
                m_t = pool.tile([128, 256], F16, tag="m", name=f"m_{grp}")
                nc.vector._custom_dve(MOP, out=m_t[:], in0=xb[:], in1=t2_t[:],
                                      s0=bias_t[:], imm2=2.0)

                if grp % 2 == 0:
                    ob = obpool.tile([128, 2048], F16, tag="ob",
                                     name=f"ob_{grp // 2}")
                # matmul dsts must start at PSUM bank boundaries: each
                # [128,1024] tile = 2 banks, outputs at cols 0 and 512.
                psA = mmpool.tile([128, 1024], F32, tag=f"mm{(2 * grp) % 3}",
                                  name=f"psA_{grp}")
                psB = mmpool.tile([128, 1024], F32,
                                  tag=f"mm{(2 * grp + 1) % 3}",
                                  name=f"psB_{grp}")
                for r in range(4):
                    ps = psA if r < 2 else psB
                    nc.tensor.matmul(
                        ps[:, (r % 2) * 512:(r % 2) * 512 + 256],
                        w_t[32 * r:32 * r + 32, 128 * grp:128 * grp + 128],
                        m_t[32 * r:32 * r + 32, :],
                        start=True, stop=True, tile_position=(32 * r, 0),
                    )
                for pair, ps in enumerate((psA, psB)):
                    off = (4 * (grp % 2) + 2 * pair) * 256
                    src = ps[:].rearrange("p (b e) -> p b e",
                                          e=512)[:, :, 0:256]
                    dst = ob[:, off:off + 512].rearrange(
                        "p (i e) -> p i e", e=256)
                    # vector carries both bump ops -> scalar takes ~54/64
                    if ncopy % 6 == 0:
                        nc.vector.tensor_copy(dst, src)
                    else:
                        nc.scalar.copy(dst, src)
                    ncopy += 1
                if grp % 2 == 1:
                    ig = 8 * (grp // 2)
                    nc.sync.dma_start(out=out_d.ap()[:, ig:ig + 8, :],
                                      in_=ob[:])
    nc.finalize()
    return nc


def _host_prep(cp):
    """cp (128,128,16) fp32 -> w_host [128, 4096] fp16 (cp_pad/6, strip
    layout), selw [128,128] fp16, bias [128,1] fp32."""
    cp_pad = np.concatenate([cp, cp[..., -1:], cp[..., -1:]], axis=-1)
    Wt = np.transpose(cp_pad, (0, 2, 1)).astype(np.float64) / 6.0  # (i, c, o)
    # w_host[32r + c, 128*grp + o] = Wt[4*grp + r, c, o]
    wh = np.zeros((4, 32, 32, 128), dtype=np.float16)  # [r, c, grp, o]
    wh[:, :NC18] = Wt.reshape(32, 4, NC18, 128).transpose(1, 2, 0, 3).astype(
        np.float16)
    w_host = wh.reshape(128, 32 * 128)

    # selw[32q + k, 128s + p] = 14 * (k // 8 == s) * ((k % 8) % 4 == p // 32)
    selw = np.zeros((128, 512), dtype=np.float16)
    k = np.arange(128) % 32                      # row within quadrant
    col = np.arange(512)
    s_col, p_col = col // 128, (col % 128) // 32  # sub-block, output strip
    sel_mask = ((k // 8)[:, None] == s_col[None, :]) & (
        ((k % 8) % 4)[:, None] == p_col[None, :])
    selw[sel_mask] = 14.0

    bias = (1.0 - np.arange(128, dtype=np.float32) % 32).reshape(128, 1)
    return w_host, selw, bias


def _make_xhm(xc):
    """xc (128, 256) fp32 -> [128, 512] fp16: group grp at rows
    32q+8s (+j: xh, +4+j: xl), cols 256*fcb."""
    xh = xc.astype(np.float16)
    xl = (xc - xh.astype(np.float32)).astype(np.float16)
    xhm = np.zeros((128, 512), dtype=np.float16)
    for grp in range(32):
        q, sblk, fcb = grp % 4, (grp // 4) % 4, grp // 16
        pr, fc = 32 * q + 8 * sblk, 256 * fcb
        xhm[pr:pr + 4, fc:fc + 256] = xh[4 * grp:4 * grp + 4]
        xhm[pr + 4:pr + 8, fc:fc + 256] = xl[4 * grp:4 * grp + 4]
    return xhm


def kernel(x, cp, k, _trace=False, _tmpdir=None):
    from concourse.bass_utils import run_bass_kernel_spmd

    x = np.asarray(x, dtype=np.float32)
    cp = np.asarray(cp, dtype=np.float32)
    assert int(k) == 3, "kernel hardcoded for cubic (k=3)"
    assert x.shape == (B, ID, NE) and cp.shape == (ID, OD, NCP)

    w_host, selw, bias = _host_prep(cp)
    in_maps = [{"w": w_host, "xhm": _make_xhm(x[c]), "selw": selw,
                "bias": bias} for c in range(NCORES)]

    if "nc" not in _cache:
        _cache["nc"] = _build_program()
    nc = _cache["nc"]

    kwargs = {}
    if _trace:
        kwargs = {"trace": True, "tmpdir": _tmpdir,
                  "trace_cores": list(range(NCORES))}
    res = run_bass_kernel_spmd(nc, in_maps, core_ids=list(range(NCORES)),
                               **kwargs)
    # device output is fp16 [o, i, e]; transpose + upcast on host
    out = np.stack([res.results[c]["out"].transpose(1, 0, 2).astype(np.float32)
                    for c in range(NCORES)], axis=0)
    if _trace:
        kernel.last_result = res
    return out


# revision 46
# speedup vs baseline: 1.0237x; 1.0237x over previous
"""Trainium2 Bass kernel for batched uniform cubic B-spline evaluation.

Reference: out[b,i,o,e] = sum_c cp_pad[i,o,c] * N_c(x[b,i,e]) where N_c is the
cardinal cubic B-spline basis on uniform knots t = arange(-3,18)/14 and cp_pad
repeats the last control point twice (c = 0..17).

Formulation used here: N_c(x) = B3(u), u = 14x + 3 - c, and with a = |u - 2|:

    6*B3 = M(a) = relu(2-a)^3 - 4*relu(1-a)^3        (no cancellation, M in [0,4])

so out[i,o,e] = sum_c (cp_pad[i,o,c]/6) * M(|14x - (c+1)|) — a single fp16
matmul per i with K=18 (padded to 32-row strips, 4 i per 128 partitions).

Per core (batch b = core id), per group of 4 i's:
  1. one K=8 fp16 matmul broadcasts 14*(xh+xl) into the 4 strips (PSUM, exact)
  2. ACT: a = Abs(xb + bias_p) with per-partition bias -(c+1); pad rows c>=18
     get a >= 5 so M = 0 automatically
  3. two 1-uop custom DVE ops: t2 = sq(relu(2*(1-a)))*(1-a) = 4*relu(1-a)^3,
     M16 = sq(relu(2-a))*(2-a) - t2 -> fp16
  4. 4 fp16 matmuls (one per i, quadrant tile_position) into PSUM banks packed
     2 i's per bank; PSUM->SBUF copies rotate vector/scalar/gpsimd
  5. batched 1MB output DMAs (8 i's each)
"""

import numpy as np

B, ID, OD, NE, NCP = 8, 128, 128, 256, 16
NCORES = 8
STRIP = 32
NC18 = 18          # control points after padding (c = 0..17)

_cache = {}
_OUT_MODE = "sbuf"          # "sbuf_per_i" (contiguous 128KB per i) or "sbuf"


def _register_dve_ops():
    """Register the two 1-uop bump ops in dve_ops' registries (idempotent)."""
    if "dve" in _cache:
        return _cache["dve"]
    import concourse.dve_ops as dve_ops
    from concourse.dve_ops import DveOp
    from concourse.dve_spec import Spec, Src0, Src1, C0, C2, Zero, relu, sq, maxx

    # a = |in0 + c0| computed inside both ops (c0 = per-partition bias AP)
    V = Src0 + C0
    A = maxx(V, Zero - V)

    def _ref_t2x(in0, in1, c0, c1, c2):
        a = np.abs(in0.astype(np.float32) + c0)
        zm = c2 - a
        return (np.maximum(zm + zm, 0) ** 2 * zm).astype(np.float32)

    zm = C2 - A
    T2 = DveOp(
        "ANT_BUMP_T2X",
        Spec(body=sq(relu(zm + zm)) * zm, reference=_ref_t2x),
        subdim=False,
        uops_sha={"v3": "bf2265f9ea4d409b"},
    )

    def _ref_mx(in0, in1, c0, c1, c2):
        a = np.abs(in0.astype(np.float32) + c0)
        z = c2 - a
        return (np.maximum(z, 0) ** 2 * z - in1).astype(np.float32)

    z = C2 - A
    M = DveOp(
        "ANT_BUMP_MX",
        Spec(body=sq(relu(z)) * z - Src1, reference=_ref_mx),
        subdim=False,
        uops_sha={"v3": "e03e3c990f1a886b"},
    )

    for op in (T2, M):
        if op.name not in dve_ops._SUB_OPCODE_FOR_NAME:
            dve_ops.OPS.append(op)
            dve_ops._SUB_OPCODE_FOR_NAME[op.name] = (
                max(dve_ops._SUB_OPCODE_FOR_NAME.values()) + 1
            )
            dve_ops.CUSTOM_DVE_SPECS[op.name] = op.spec
    assert max(dve_ops._SUB_OPCODE_FOR_NAME.values()) < 0x20
    _cache["dve"] = (T2, M)
    return T2, M


def _build_program():
    import concourse.mybir as mybir
    import concourse.tile as tile
    from concourse import bacc

    T2OP, MOP = _register_dve_ops()

    F32 = mybir.dt.float32
    F16 = mybir.dt.float16
    ABS = mybir.ActivationFunctionType.Abs

    from concourse.alu_op_type import AluOpType

    nc = bacc.Bacc("TRN2", target_bir_lowering=False)
    w_d = nc.dram_tensor("w", [128, 32 * 128], F16, kind="ExternalInput")
    xhm_d = nc.dram_tensor("xhm", [128, 512], F16, kind="ExternalInput")
    selw_d = nc.dram_tensor("selw", [128, 512], F16, kind="ExternalInput")
    bias_d = nc.dram_tensor("bias", [128, 1], F32, kind="ExternalInput")
    # fp16 output in [o, i, e] layout: halves write traffic and gives 4KB
    # contiguous DMA descriptors; host transposes + upcasts to fp32.
    out_d = nc.dram_tensor("out", [128, 128, 256], F16, kind="ExternalOutput")

    with tile.TileContext(nc) as tc:
        with (
            tc.tile_pool(name="const", bufs=1) as cpool,
            tc.tile_pool(name="work", bufs=5) as pool,
            tc.tile_pool(name="obp", bufs=4) as obpool,
            tc.tile_pool(name="xbp", bufs=1, space="PSUM") as xbpool,
            tc.tile_pool(name="mmp", bufs=1, space="PSUM") as mmpool,
        ):
            # few big DMAs: SP dispatch is ~600ns per dma_start
            selw_t = cpool.tile([128, 512], F16)
            nc.sync.dma_start(out=selw_t[:], in_=selw_d.ap())
            bias_t = cpool.tile([128, 1], F32)
            nc.sync.dma_start(out=bias_t[:], in_=bias_d.ap())
            xhm_t = cpool.tile([128, 512], F16)
            nc.sync.dma_start(out=xhm_t[:], in_=xhm_d.ap())
            w_t = cpool.tile([128, 32 * 128], F16)
            for wc in range(8):
                nc.sync.dma_start(out=w_t[:, wc * 512:(wc + 1) * 512],
                                  in_=w_d.ap()[:, wc * 512:(wc + 1) * 512])

            ncopy = 0
            ob = None
            pend_v = []   # vector copies deferred one group (in-order hazard)
            for grp in range(32):
                q, s, fcb = grp % 4, (grp // 4) % 4, grp // 16
                pr, fc = 32 * q + 8 * s, 256 * fcb

                xb = xbpool.tile([128, 256], F32, tag=f"xb{grp % 2}",
                                 name=f"xb_{grp}")
                nc.tensor.matmul(
                    xb[:], selw_t[32 * q:32 * q + 32, 128 * s:128 * s + 128],
                    xhm_t[32 * q:32 * q + 32, fc:fc + 256],
                    start=True, stop=True, tile_position=(32 * q, 0),
                )
                t2_t = pool.tile([128, 256], F32, tag="t2", name=f"t2_{grp}")
                nc.vector._custom_dve(T2OP, out=t2_t[:], in0=xb[:],
                                      s0=bias_t[:], imm2=1.0)
                m_t = pool.tile([128, 256], F16, tag="m", name=f"m_{grp}")
                nc.vector._custom_dve(MOP, out=m_t[:], in0=xb[:], in1=t2_t[:],
                                      s0=bias_t[:], imm2=2.0)
                for dst, src in pend_v:
                    nc.vector.tensor_copy(dst, src)
                pend_v.clear()

                if grp % 2 == 0:
                    ob = obpool.tile([128, 2048], F16, tag="ob",
                                     name=f"ob_{grp // 2}")
                # matmul dsts must start at PSUM bank boundaries: each
                # [128,1024] tile = 2 banks, outputs at cols 0 and 512.
                psA = mmpool.tile([128, 1024], F32, tag=f"mm{(2 * grp) % 3}",
                                  name=f"psA_{grp}")
                psB = mmpool.tile([128, 1024], F32,
                                  tag=f"mm{(2 * grp + 1) % 3}",
                                  name=f"psB_{grp}")
                for r in range(4):
                    ps = psA if r < 2 else psB
                    nc.tensor.matmul(
                        ps[:, (r % 2) * 512:(r % 2) * 512 + 256],
                        w_t[32 * r:32 * r + 32, 128 * grp:128 * grp + 128],
                        m_t[32 * r:32 * r + 32, :],
                        start=True, stop=True, tile_position=(32 * r, 0),
                    )
                for pair, ps in enumerate((psA, psB)):
                    off = (4 * (grp % 2) + 2 * pair) * 256
                    src = ps[:].rearrange("p (b e) -> p b e",
                                          e=512)[:, :, 0:256]
                    dst = ob[:, off:off + 512].rearrange(
                        "p (i e) -> p i e", e=256)
                    # vector carries both bump ops -> scalar takes ~54/64;
                    # vector's share is deferred to after the next group's
                    # bump ops so it never blocks them in the queue
                    if ncopy % 6 == 0:
                        pend_v.append((dst, src))
                    else:
                        nc.scalar.copy(dst, src)
                    ncopy += 1
                if grp % 2 == 1:
                    for dst, src in pend_v:
                        nc.vector.tensor_copy(dst, src)
                    pend_v.clear()
                    ig = 8 * (grp // 2)
                    nc.sync.dma_start(out=out_d.ap()[:, ig:ig + 8, :],
                                      in_=ob[:])
    nc.finalize()
    return nc


def _host_prep(cp):
    """cp (128,128,16) fp32 -> w_host [128, 4096] fp16 (cp_pad/6, strip
    layout), selw [128,128] fp16, bias [128,1] fp32."""
    cp_pad = np.concatenate([cp, cp[..., -1:], cp[..., -1:]], axis=-1)
    Wt = np.transpose(cp_pad, (0, 2, 1)).astype(np.float64) / 6.0  # (i, c, o)
    # w_host[32r + c, 128*grp + o] = Wt[4*grp + r, c, o]
    wh = np.zeros((4, 32, 32, 128), dtype=np.float16)  # [r, c, grp, o]
    wh[:, :NC18] = Wt.reshape(32, 4, NC18, 128).transpose(1, 2, 0, 3).astype(
        np.float16)
    w_host = wh.reshape(128, 32 * 128)

    # selw[32q + k, 128s + p] = 14 * (k // 8 == s) * ((k % 8) % 4 == p // 32)
    selw = np.zeros((128, 512), dtype=np.float16)
    k = np.arange(128) % 32                      # row within quadrant
    col = np.arange(512)
    s_col, p_col = col // 128, (col % 128) // 32  # sub-block, output strip
    sel_mask = ((k // 8)[:, None] == s_col[None, :]) & (
        ((k % 8) % 4)[:, None] == p_col[None, :])
    selw[sel_mask] = 14.0

    bias = (1.0 - np.arange(128, dtype=np.float32) % 32).reshape(128, 1)
    return w_host, selw, bias


def _make_xhm(xc):
    """xc (128, 256) fp32 -> [128, 512] fp16: group grp at rows
    32q+8s (+j: xh, +4+j: xl), cols 256*fcb."""
    xh = xc.astype(np.float16)
    xl = (xc - xh.astype(np.float32)).astype(np.float16)
    xhm = np.zeros((128, 512), dtype=np.float16)
    for grp in range(32):
        q, sblk, fcb = grp % 4, (grp // 4) % 4, grp // 16
        pr, fc = 32 * q + 8 * sblk, 256 * fcb
        xhm[pr:pr + 4, fc:fc + 256] = xh[4 * grp:4 * grp + 4]
        xhm[pr + 4:pr + 8, fc:fc + 256] = xl[4 * grp:4 * grp + 4]
    return xhm


def kernel(x, cp, k, _trace=False, _tmpdir=None):
    from concourse.bass_utils import run_bass_kernel_spmd

    x = np.asarray(x, dtype=np.float32)
    cp = np.asarray(cp, dtype=np.float32)
    assert int(k) == 3, "kernel hardcoded for cubic (k=3)"
    assert x.shape == (B, ID, NE) and cp.shape == (ID, OD, NCP)

    w_host, selw, bias = _host_prep(cp)
    in_maps = [{"w": w_host, "xhm": _make_xhm(x[c]), "selw": selw,
                "bias": bias} for c in range(NCORES)]

    if "nc" not in _cache:
        _cache["nc"] = _build_program()
    nc = _cache["nc"]

    kwargs = {}
    if _trace:
        kwargs = {"trace": True, "tmpdir": _tmpdir,
                  "trace_cores": list(range(NCORES))}
    res = run_bass_kernel_spmd(nc, in_maps, core_ids=list(range(NCORES)),
                               **kwargs)
    # device output is fp16 [o, i, e]; transpose + upcast on host
    out = np.stack([res.results[c]["out"].transpose(1, 0, 2).astype(np.float32)
                    for c in range(NCORES)], axis=0)
    if _trace:
        kernel.last_result = res
    return out


# revision 55
# speedup vs baseline: 1.1494x; 1.1227x over previous
"""Trainium2 Bass kernel for batched uniform cubic B-spline evaluation.

Reference: out[b,i,o,e] = sum_c cp_pad[i,o,c] * N_c(x[b,i,e]) where N_c is the
cardinal cubic B-spline basis on uniform knots t = arange(-3,18)/14 and cp_pad
repeats the last control point twice (c = 0..17).

Formulation used here: N_c(x) = B3(u), u = 14x + 3 - c, and with a = |u - 2|:

    6*B3 = M(a) = relu(2-a)^3 - 4*relu(1-a)^3        (no cancellation, M in [0,4])

so out[i,o,e] = sum_c (cp_pad[i,o,c]/6) * M(|14x - (c+1)|) — a single fp16
matmul per i with K=18 (padded to 32-row strips, 4 i per 128 partitions).

Per core (batch b = core id), per group of 4 i's:
  1. one K=8 fp16 matmul broadcasts 14*(xh+xl) into the 4 strips (PSUM, exact)
  2. ACT: a = Abs(xb + bias_p) with per-partition bias -(c+1); pad rows c>=18
     get a >= 5 so M = 0 automatically
  3. two 1-uop custom DVE ops: t2 = sq(relu(2*(1-a)))*(1-a) = 4*relu(1-a)^3,
     M16 = sq(relu(2-a))*(2-a) - t2 -> fp16
  4. 4 fp16 matmuls (one per i, quadrant tile_position) into PSUM banks packed
     2 i's per bank; PSUM->SBUF copies rotate vector/scalar/gpsimd
  5. batched 1MB output DMAs (8 i's each)
"""

import numpy as np

B, ID, OD, NE, NCP = 8, 128, 128, 256, 16
NCORES = 8
STRIP = 32
NC18 = 18          # control points after padding (c = 0..17)

_cache = {}
_OUT_MODE = "sbuf"          # "sbuf_per_i" (contiguous 128KB per i) or "sbuf"


def _register_dve_ops():
    """Register the two 1-uop bump ops in dve_ops' registries (idempotent)."""
    if "dve" in _cache:
        return _cache["dve"]
    import concourse.dve_ops as dve_ops
    from concourse.dve_ops import DveOp
    from concourse.dve_spec import Spec, Src0, C0, C1, Zero, relu, sq, maxx

    # out = relu(K - |in0 + bias|)^3 with per-partition bias (c0) and K (c1):
    # K=2 rows give relu(2-a)^3 taps, K=1 rows give relu(1-a)^3 taps; the
    # -4x weight of the second cube is folded into the matmul weights.
    V = Src0 + C0
    A = maxx(V, Zero - V)
    Z = C1 - A

    def _ref_b3(in0, in1, c0, c1, c2):
        a = np.abs(in0.astype(np.float32) + c0)
        z = c1 - a
        return (np.maximum(z, 0) ** 2 * z).astype(np.float32)

    B3 = DveOp(
        "ANT_BUMP3",
        Spec(body=sq(relu(Z)) * Z, reference=_ref_b3),
        subdim=False,
        uops_sha={"v3": "d04e0e801d602ffa"},
    )

    if B3.name not in dve_ops._SUB_OPCODE_FOR_NAME:
        dve_ops.OPS.append(B3)
        dve_ops._SUB_OPCODE_FOR_NAME[B3.name] = (
            max(dve_ops._SUB_OPCODE_FOR_NAME.values()) + 1
        )
        dve_ops.CUSTOM_DVE_SPECS[B3.name] = B3.spec
    assert max(dve_ops._SUB_OPCODE_FOR_NAME.values()) < 0x20
    _cache["dve"] = B3
    return B3


def _build_program():
    import concourse.mybir as mybir
    import concourse.tile as tile
    from concourse import bacc

    B3OP = _register_dve_ops()

    F32 = mybir.dt.float32
    F16 = mybir.dt.float16
    ABS = mybir.ActivationFunctionType.Abs

    from concourse.alu_op_type import AluOpType

    nc = bacc.Bacc("TRN2", target_bir_lowering=False)
    w_d = nc.dram_tensor("w", [128, 32 * 128], F16, kind="ExternalInput")
    xhm_d = nc.dram_tensor("xhm", [128, 512], F16, kind="ExternalInput")
    selw_d = nc.dram_tensor("selw", [128, 512], F16, kind="ExternalInput")
    bias_d = nc.dram_tensor("bias", [128, 1], F32, kind="ExternalInput")
    kap_d = nc.dram_tensor("kap", [128, 1], F32, kind="ExternalInput")
    # fp16 output in [o, i, e] layout: halves write traffic and gives 4KB
    # contiguous DMA descriptors; host transposes + upcasts to fp32.
    out_d = nc.dram_tensor("out", [128, 128, 256], F16, kind="ExternalOutput")

    with tile.TileContext(nc) as tc:
        with (
            tc.tile_pool(name="const", bufs=1) as cpool,
            tc.tile_pool(name="work", bufs=5) as pool,
            tc.tile_pool(name="obp", bufs=4) as obpool,
            tc.tile_pool(name="xbp", bufs=1, space="PSUM") as xbpool,
            tc.tile_pool(name="mmp", bufs=1, space="PSUM") as mmpool,
        ):
            # few big DMAs: SP dispatch is ~600ns per dma_start
            selw_t = cpool.tile([128, 512], F16)
            nc.sync.dma_start(out=selw_t[:], in_=selw_d.ap())
            bias_t = cpool.tile([128, 1], F32)
            nc.sync.dma_start(out=bias_t[:], in_=bias_d.ap())
            kap_t = cpool.tile([128, 1], F32)
            nc.sync.dma_start(out=kap_t[:], in_=kap_d.ap())
            xhm_t = cpool.tile([128, 512], F16)
            nc.sync.dma_start(out=xhm_t[:], in_=xhm_d.ap())
            w_t = cpool.tile([128, 32 * 128], F16)
            for wc in range(8):
                nc.sync.dma_start(out=w_t[:, wc * 512:(wc + 1) * 512],
                                  in_=w_d.ap()[:, wc * 512:(wc + 1) * 512])

            ncopy = 0
            ob = None
            pend_v = []   # vector copies deferred one group (in-order hazard)
            for grp in range(32):
                q, s, fcb = grp % 4, (grp // 4) % 4, grp // 16
                pr, fc = 32 * q + 8 * s, 256 * fcb

                xb = xbpool.tile([128, 256], F32, tag=f"xb{grp % 2}",
                                 name=f"xb_{grp}")
                nc.tensor.matmul(
                    xb[:], selw_t[32 * q:32 * q + 32, 128 * s:128 * s + 128],
                    xhm_t[32 * q:32 * q + 32, fc:fc + 256],
                    start=True, stop=True, tile_position=(32 * q, 0),
                )
                m_t = pool.tile([128, 256], F16, tag="m", name=f"m_{grp}")
                nc.vector._custom_dve(B3OP, out=m_t[:], in0=xb[:],
                                      s0=bias_t[:], s1=kap_t[:])
                for dst, src in pend_v:
                    nc.vector.tensor_copy(dst, src)
                pend_v.clear()

                if grp % 2 == 0:
                    ob = obpool.tile([128, 2048], F16, tag="ob",
                                     name=f"ob_{grp // 2}")
                # matmul dsts must start at PSUM bank boundaries: each
                # [128,1024] tile = 2 banks, outputs at cols 0 and 512.
                psA = mmpool.tile([128, 1024], F32, tag=f"mm{(2 * grp) % 3}",
                                  name=f"psA_{grp}")
                psB = mmpool.tile([128, 1024], F32,
                                  tag=f"mm{(2 * grp + 1) % 3}",
                                  name=f"psB_{grp}")
                for r in range(4):
                    ps = psA if r < 2 else psB
                    nc.tensor.matmul(
                        ps[:, (r % 2) * 512:(r % 2) * 512 + 256],
                        w_t[32 * r:32 * r + 32, 128 * grp:128 * grp + 128],
                        m_t[32 * r:32 * r + 32, :],
                        start=True, stop=True, tile_position=(32 * r, 0),
                    )
                for pair, ps in enumerate((psA, psB)):
                    off = (4 * (grp % 2) + 2 * pair) * 256
                    src = ps[:].rearrange("p (b e) -> p b e",
                                          e=512)[:, :, 0:256]
                    dst = ob[:, off:off + 512].rearrange(
                        "p (i e) -> p i e", e=256)
                    # vector carries the bump op -> scalar takes ~2/3 of
                    # copies; vector's share is deferred past the next
                    # group's bump op so it never blocks it in the queue
                    if ncopy % 3 == 0:
                        pend_v.append((dst, src))
                    else:
                        nc.scalar.copy(dst, src)
                    ncopy += 1
                if grp % 2 == 1:
                    for dst, src in pend_v:
                        nc.vector.tensor_copy(dst, src)
                    pend_v.clear()
                    ig = 8 * (grp // 2)
                    nc.sync.dma_start(out=out_d.ap()[:, ig:ig + 8, :],
                                      in_=ob[:])
    nc.finalize()
    return nc


def _host_prep(cp):
    """cp (128,128,16) fp32 -> w_host [128, 4096] fp16 (cp_pad/6, strip
    layout), selw [128,128] fp16, bias [128,1] fp32."""
    cp_pad = np.concatenate([cp, cp[..., -1:], cp[..., -1:]], axis=-1)
    # strip row rr < 17: t1 tap c=rr, weight cp_pad/6, K=2, bias=1-c
    # strip row rr >= 17: t2 tap c=rr-16, weight -(2/3)cp_pad, K=1, bias=1-c
    Wrow = np.empty((128, 32, 128))  # (i, rr, o)
    Wrow[:, :17, :] = np.transpose(cp_pad[:, :, 0:17], (0, 2, 1)) / 6.0
    Wrow[:, 17:, :] = -np.transpose(cp_pad[:, :, 1:16], (0, 2, 1)) * (2.0 / 3.0)
    # w_host[32r + rr, 128*grp + o] = Wrow[4*grp + r, rr, o]
    w_host = (Wrow.reshape(32, 4, 32, 128).transpose(1, 2, 0, 3)
              .astype(np.float16).reshape(128, 32 * 128))

    # selw[32q + k, 128s + p] = 14 * (k // 8 == s) * ((k % 8) % 4 == p // 32)
    selw = np.zeros((128, 512), dtype=np.float16)
    k = np.arange(128) % 32                      # row within quadrant
    col = np.arange(512)
    s_col, p_col = col // 128, (col % 128) // 32  # sub-block, output strip
    sel_mask = ((k // 8)[:, None] == s_col[None, :]) & (
        ((k % 8) % 4)[:, None] == p_col[None, :])
    selw[sel_mask] = 14.0

    rr = np.arange(128, dtype=np.float32) % 32
    bias = np.where(rr < 17, 1.0 - rr, 17.0 - rr).astype(np.float32)
    bias = bias.reshape(128, 1)
    kap = np.where(rr < 17, 2.0, 1.0).astype(np.float32).reshape(128, 1)
    return w_host, selw, bias, kap


def _make_xhm(xc):
    """xc (128, 256) fp32 -> [128, 512] fp16: group grp at rows
    32q+8s (+j: xh, +4+j: xl), cols 256*fcb."""
    xh = xc.astype(np.float16)
    xl = (xc - xh.astype(np.float32)).astype(np.float16)
    xhm = np.zeros((128, 512), dtype=np.float16)
    for grp in range(32):
        q, sblk, fcb = grp % 4, (grp // 4) % 4, grp // 16
        pr, fc = 32 * q + 8 * sblk, 256 * fcb
        xhm[pr:pr + 4, fc:fc + 256] = xh[4 * grp:4 * grp + 4]
        xhm[pr + 4:pr + 8, fc:fc + 256] = xl[4 * grp:4 * grp + 4]
    return xhm


def kernel(x, cp, k, _trace=False, _tmpdir=None):
    from concourse.bass_utils import run_bass_kernel_spmd

    x = np.asarray(x, dtype=np.float32)
    cp = np.asarray(cp, dtype=np.float32)
    assert int(k) == 3, "kernel hardcoded for cubic (k=3)"
    assert x.shape == (B, ID, NE) and cp.shape == (ID, OD, NCP)

    w_host, selw, bias, kap = _host_prep(cp)
    in_maps = [{"w": w_host, "xhm": _make_xhm(x[c]), "selw": selw,
                "bias": bias, "kap": kap} for c in range(NCORES)]

    if "nc" not in _cache:
        _cache["nc"] = _build_program()
    nc = _cache["nc"]

    kwargs = {}
    if _trace:
        kwargs = {"trace": True, "tmpdir": _tmpdir,
                  "trace_cores": list(range(NCORES))}
    res = run_bass_kernel_spmd(nc, in_maps, core_ids=list(range(NCORES)),
                               **kwargs)
    # device output is fp16 [o, i, e]; transpose + upcast on host
    out = np.stack([res.results[c]["out"].transpose(1, 0, 2).astype(np.float32)
                    for c in range(NCORES)], axis=0)
    if _trace:
        kernel.last_result = res
    return out
